# revision 3
# baseline (speedup 1.0000x reference)
"""ARD RBF Gram matrix kernel for Trainium2 (8 NeuronCores, SPMD) — v10.

K[i, j] = exp(-0.5 * sum_d (x[i,d] - y[j,d])^2 / exp(logh[d]))

Sharding: 2x4 core grid; core (r, q) owns rows [r*4096, ...) x cols
[q*2048, ...). Host stages x/y as fp16 (input-precision choice; fp16's
5e-4 error is negligible next to the fp8e4 matmul operands). Per-core
HBM: 6MB in, 16.8MB fp16 out.

Pipeline design (what each ring/engine owns):
  - ih^2 scaling lives on the y side only: ys8 = fp8(-2*e^-logh * y)
    (ACT, one 2048-wide op per chunk), xs8 = fp8(x) pure cast — fp8
    relative error is scale-invariant, so moving the scale is free.
  - sync HWDGE ring, strictly FIFO: y chunks, first 3 x-slab fp16
    loads, the slab-0 x2 bounce transpose, then interleaved [later
    x-slab loads | x2 bounces | output stores] in itile order. The ring
    is kept shallow so early transfers are never queued behind bulk.
  - SWDGE ring (gpsimd): fp16->fp8 SBUF->SBUF casts xstage -> xs8, one
    per 512-col slab, emitted two windows ahead of use.
  - scalar HWDGE ring: unused (keeps the ACT FIFO free of DMA waits).
  - x2 bias (-0.5*sum e^-logh x^2): DVE square of the fp16 slab (2x
    mode), 4 accumulating PE matmuls into a 2-bank PSUM ring, DVE row
    copy, DRAM-bounce transpose [1,512] -> [128,4].
  - ey2rep (exp(-0.5*y2) replicated to 128 partitions): one 512-wide js
    piece per itile over itiles 1-4: PE reduce -> DVE bf16 row copy ->
    ones-matmul into a main PSUM tile -> ACT exp straight into the
    fp16 [P, MJ] tile. Multiplies for itiles 0-4 are deferred until the
    last piece is emitted (program order defines dependencies).
  - Main loop per itile: 8 fp8 DoubleRow matmuls (contraction-pair
    outer) into two [P,1024] tiles of a 3-deep PSUM ring, ACT exp per
    half (bias = -0.5*x2[i], scale = -0.5), DVE fp16 multiply by
    ey2rep, store.

Host code only reshapes/transposes/shards numpy arrays, picks the fp16
staging precision, and losslessly widens the fp16 result.
"""

import json

import numpy as np

import concourse.bass as bass
import concourse.mybir as mybir
import concourse.tile as tile
from concourse.bass_utils import run_bass_kernel_spmd

N_CORES = 8
N, M, D = 8192, 8192, 512
RG, CG = 2, 4
NI = N // RG  # 4096
MJ = M // CG  # 2048
P = 128
NCHUNK = D // P  # 4
NPAIR = NCHUNK // 2  # 2
ITILES = NI // P  # 32
SLABW = 512
NSLAB = NI // SLABW  # 8
IPS = SLABW // P  # 4
HW = MJ // 2  # 1024

F32 = mybir.dt.float32
BF16 = mybir.dt.bfloat16
F16 = mybir.dt.float16
FP8 = mybir.dt.float8e4
AF = mybir.ActivationFunctionType
DR = mybir.MatmulPerfMode.DoubleRow

_WAIT_LIMIT = 1


def _split_excess_waits(bir: dict, limit: int = _WAIT_LIMIT) -> dict:
    counter = 0
    for fn in bir.get("functions", []):
        for bb in fn.get("blocks", []):
            new_insts = []
            for inst in bb.get("instructions", []):
                si = inst.get("sync_info")
                waits = si.get("on_wait") if si else None
                eng = inst.get("engine", "Unassigned")
                if waits and len(waits) > limit and eng != "Unassigned":
                    keep = len(waits) % 2
                    head = waits[: len(waits) - keep]
                    for i in range(0, len(head), 2):
                        counter += 1
                        new_insts.append(
                            {
                                "debug": inst.get("debug", 0),
                                "engine": eng,
                                "ins": [],
                                "outs": [],
                                "name": f"WS-{counter}-{inst['name']}",
                                "opcode": "EventSemaphore",
                                "sync_info": {
                                    "on_update": [],
                                    "on_wait": head[i : i + 2],
                                },
                            }
                        )
                    si["on_wait"] = waits[len(waits) - keep :]
                new_insts.append(inst)
            bb["instructions"] = new_insts
    return bir


def _patch_nc(nc):
    orig = nc.to_json_bytes

    def patched() -> bytes:
        return json.dumps(_split_excess_waits(json.loads(orig()))).encode()

    nc.to_json_bytes = patched
    return nc


def _build_nc():
    nc = bass.Bass()

    xt = nc.dram_tensor("xt", [D, NI], F16, kind="ExternalInput")
    yt = nc.dram_tensor("yt", [D, MJ], F16, kind="ExternalInput")
    lh = nc.dram_tensor("lh", [NCHUNK, P], F32, kind="ExternalInput")
    out = nc.dram_tensor("out", [NI, MJ], F16, kind="ExternalOutput")
    scratch = nc.dram_tensor("scratch", [NSLAB, SLABW], F32, kind="Internal")

    xt_r = xt.rearrange("(c d) i -> d c i", d=P)
    yt_r = yt.rearrange("(c d) j -> d c j", d=P)

    with tile.TileContext(nc) as tc:
        with (
            tc.tile_pool(name="singles", bufs=1) as singles,
            tc.tile_pool(name="sqyp", bufs=4) as sqyp,
            tc.tile_pool(name="sqxp", bufs=2) as sqxp,
            tc.tile_pool(name="sxp", bufs=2) as sxp,
            tc.tile_pool(name="yrowp", bufs=2) as yrowp,
            tc.tile_pool(name="tmpp", bufs=8) as tmpp,
            tc.tile_pool(name="outp", bufs=10) as outp,
            tc.tile_pool(name="psp", bufs=3, space="PSUM") as psp,
            tc.tile_pool(name="accp", bufs=2, space="PSUM") as accp,
        ):
            xs8 = singles.tile([P, NCHUNK, NI], FP8)
            ys8 = singles.tile([P, NCHUNK, MJ], FP8)
            mhx2 = singles.tile([P, ITILES], F32)
            ey2rep = singles.tile([P, MJ], F16)
            ones1 = singles.tile([1, P], BF16)
            lhs = singles.tile([P, NCHUNK], F32)
            emlh = singles.tile([P, NCHUNK], F32)
            ihm2 = singles.tile([P, NCHUNK], F32)
            mihsq = singles.tile([P, NCHUNK], BF16)
            ident = singles.tile([1, 1], F32)
            xstage = [
                singles.tile([P, NCHUNK * SLABW], F16, name=f"xst{s}")
                for s in range(NSLAB)
            ]
            ystage = [
                singles.tile([P, MJ], F16, name=f"yst{c}")
                for c in range(NCHUNK)
            ]

            # consts
            nc.sync.dma_start(out=lhs, in_=lh.rearrange("c d -> d c"))
            nc.scalar.activation(emlh, lhs, AF.Exp, scale=-1.0)
            nc.scalar.mul(ihm2, emlh, -2.0)
            nc.scalar.mul(mihsq, emlh, -0.5)
            nc.vector.memset(ones1, 1.0)
            nc.vector.memset(ident, 1.0)

            # head loads, shallow: y chunks then only the first 3 x slabs
            for c in range(NCHUNK):
                nc.sync.dma_start(out=ystage[c], in_=yt_r[:, c, :])
            for s in range(3):
                nc.sync.dma_start(
                    out=xstage[s], in_=xt_r[:, :, s * SLABW : (s + 1) * SLABW]
                )

            def cast_slab(s):
                nc.gpsimd.dma_start(
                    out=xs8[:, :, s * SLABW : (s + 1) * SLABW], in_=xstage[s]
                )

            cast_slab(0)
            cast_slab(1)

            # x slab 0 prep: fp16 square (2x mode), reduce, row copy
            sqx = sqxp.tile([P, NCHUNK * SLABW], BF16, tag="sx", name="sqx0")
            nc.vector.tensor_mul(sqx, xstage[0], xstage[0])
            xacc = accp.tile([1, SLABW], F32, tag="a", name="xa0")
            for c in range(NCHUNK):
                nc.tensor.matmul(
                    xacc,
                    mihsq[:, c : c + 1],
                    sqx[:, c * SLABW : (c + 1) * SLABW],
                    start=(c == 0),
                    stop=(c == NCHUNK - 1),
                )
            sx0 = sxp.tile([1, SLABW], F32, tag="r", name="sx0")
            nc.vector.tensor_copy(sx0, xacc)
            mht = psp.tile([P, HW], F32, tag="ps", name="mht0")
            for t in range(IPS):
                nc.tensor.transpose(
                    mht[:, t : t + 1], sx0[0:1, t * P : (t + 1) * P], ident
                )
            nc.vector.tensor_copy(mhx2[:, 0:IPS], mht[:, 0:IPS])

            # y prep: squares (DVE) + fp8 conversions (ACT)
            sqys = []
            for c in range(NCHUNK):
                sqy = sqyp.tile([P, MJ], BF16, tag="sq", name=f"sqy{c}")
                nc.vector.tensor_mul(sqy, ystage[c], ystage[c])
                sqys.append(sqy)
                nc.scalar.mul(ys8[:, c, :], ystage[c], ihm2[:, c : c + 1])

            # ---- main loop ----
            pending = []
            for it in range(ITILES):
                w = it // IPS
                k = it % IPS
                if k == 0:
                    if w + 2 < NSLAB:
                        cast_slab(w + 2)
                    if w + 3 < NSLAB:
                        nc.sync.dma_start(
                            out=xstage[w + 3],
                            in_=xt_r[:, :, (w + 3) * SLABW : (w + 4) * SLABW],
                        )
                s = w + 1
                if s < NSLAB:
                    s0 = s * SLABW
                    if k == 0:
                        sqx = sqxp.tile(
                            [P, NCHUNK * SLABW], BF16, tag="sx", name=f"sqx{s}"
                        )
                        nc.vector.tensor_mul(sqx, xstage[s], xstage[s])
                    elif k == 1:
                        xacc = accp.tile([1, SLABW], F32, tag="a", name=f"xa{s}")
                        for c in range(NCHUNK):
                            nc.tensor.matmul(
                                xacc,
                                mihsq[:, c : c + 1],
                                sqx[:, c * SLABW : (c + 1) * SLABW],
                                start=(c == 0),
                                stop=(c == NCHUNK - 1),
                            )
                        sxr = sxp.tile([1, SLABW], F32, tag="r", name=f"sx{s}")
                        nc.vector.tensor_copy(sxr, xacc)
                    elif k == 2:
                        mht = psp.tile([P, HW], F32, tag="ps", name=f"mht{s}")
                        for t in range(IPS):
                            nc.tensor.transpose(
                                mht[:, t : t + 1],
                                sxr[0:1, t * P : (t + 1) * P],
                                ident,
                            )
                        nc.vector.tensor_copy(
                            mhx2[:, s * IPS : (s + 1) * IPS], mht[:, 0:IPS]
                        )

                isl = slice(it * P, (it + 1) * P)
                tmp = tmpp.tile([P, MJ], F16, tag="tmp", name=f"t{it}")
                ps0 = psp.tile([P, HW], F32, tag="ps", name=f"ps{it}_0")
                ps1 = psp.tile([P, HW], F32, tag="ps", name=f"ps{it}_1")
                pss = (ps0, ps1)
                for t in range(NPAIR):
                    csl = slice(2 * t, 2 * t + 2)
                    for pos in range(4):
                        h, js = divmod(pos, 2)
                        nc.tensor.matmul(
                            pss[h][:, js * 512 : (js + 1) * 512],
                            xs8[:, csl, isl],
                            ys8[:, csl, pos * 512 : (pos + 1) * 512],
                            start=(t == 0),
                            stop=(t == NPAIR - 1),
                            perf_mode=DR,
                        )
                for h in range(2):
                    nc.scalar.activation(
                        tmp[:, h * HW : (h + 1) * HW],
                        pss[h],
                        AF.Exp,
                        bias=mhx2[:, it : it + 1],
                        scale=-0.5,
                    )

                # ey2rep piece js = it-1 over itiles 1..4 (PE/DVE/ACT filler)
                if 1 <= it <= MJ // 512:
                    js = it - 1
                    jsl = slice(js * 512, (js + 1) * 512)
                    yacc = accp.tile([1, 512], F32, tag="a", name=f"ya{js}")
                    for c in range(NCHUNK):
                        nc.tensor.matmul(
                            yacc,
                            mihsq[:, c : c + 1],
                            sqys[c][:, jsl],
                            start=(c == 0),
                            stop=(c == NCHUNK - 1),
                        )
                    yrow = yrowp.tile([1, 512], BF16, tag="yr", name=f"yr{js}")
                    nc.vector.tensor_copy(yrow, yacc)
                    rep = psp.tile([P, HW], F32, tag="ps", name=f"rep{js}")
                    nc.tensor.matmul(
                        rep[:, 0:512], ones1, yrow, start=True, stop=True
                    )
                    nc.scalar.activation(ey2rep[:, jsl], rep[:, 0:512], AF.Exp)

                # multiplies/stores; itiles 0-4 deferred until the last
                # ey2rep piece's writers exist in program order
                pending.append((tmp, isl, it))
                if it >= MJ // 512:
                    for ptmp, pisl, pit in pending:
                        ot = outp.tile([P, MJ], F16, tag="ot", name=f"ot{pit}")
                        nc.vector.tensor_mul(ot, ptmp, ey2rep)
                        nc.sync.dma_start(out=out[pisl, :], in_=ot)
                    pending.clear()

    return _patch_nc(nc)


_NC_CACHE = None
_TRACE = False
_TRACE_KWARGS = {}
LAST_RESULTS = None


def kernel(x, y, logh):
    global _NC_CACHE, LAST_RESULTS
    x = np.ascontiguousarray(np.asarray(x, dtype=np.float32))
    y = np.ascontiguousarray(np.asarray(y, dtype=np.float32))
    logh = np.ascontiguousarray(np.asarray(logh, dtype=np.float32))
    assert x.shape == (N, D) and y.shape == (M, D) and logh.shape == (D,)

    if _NC_CACHE is None:
        _NC_CACHE = _build_nc()
    nc = _NC_CACHE

    lhm = np.ascontiguousarray(logh.reshape(NCHUNK, P))
    x16 = x.astype(np.float16)
    y16 = y.astype(np.float16)
    xts = [
        np.ascontiguousarray(x16[r * NI : (r + 1) * NI, :].T)
        for r in range(RG)
    ]
    yts = [
        np.ascontiguousarray(y16[q * MJ : (q + 1) * MJ, :].T)
        for q in range(CG)
    ]
    in_maps = []
    for c in range(N_CORES):
        r, q = divmod(c, CG)
        in_maps.append({"xt": xts[r], "yt": yts[q], "lh": lhm})

    res = run_bass_kernel_spmd(
        nc,
        in_maps,
        core_ids=list(range(N_CORES)),
        trace=_TRACE,
        **_TRACE_KWARGS,
    )
    LAST_RESULTS = res
    full = np.empty((N, M), dtype=np.float32)
    for c in range(N_CORES):
        r, q = divmod(c, CG)
        full[r * NI : (r + 1) * NI, q * MJ : (q + 1) * MJ] = res.results[c][
            "out"
        ].astype(np.float32)
    return full


# revision 4
# speedup vs baseline: 1.0089x; 1.0089x over previous
"""ARD RBF Gram matrix kernel for Trainium2 (8 NeuronCores, SPMD) — v10.

K[i, j] = exp(-0.5 * sum_d (x[i,d] - y[j,d])^2 / exp(logh[d]))

Sharding: 2x4 core grid; core (r, q) owns rows [r*4096, ...) x cols
[q*2048, ...). Host stages x/y as fp16 (input-precision choice; fp16's
5e-4 error is negligible next to the fp8e4 matmul operands). Per-core
HBM: 6MB in, 16.8MB fp16 out.

Pipeline design (what each ring/engine owns):
  - ih^2 scaling lives on the y side only: ys8 = fp8(-2*e^-logh * y)
    (ACT, one 2048-wide op per chunk), xs8 = fp8(x) pure cast — fp8
    relative error is scale-invariant, so moving the scale is free.
  - sync HWDGE ring, strictly FIFO: y chunks, first 3 x-slab fp16
    loads, the slab-0 x2 bounce transpose, then interleaved [later
    x-slab loads | x2 bounces | output stores] in itile order. The ring
    is kept shallow so early transfers are never queued behind bulk.
  - SWDGE ring (gpsimd): fp16->fp8 SBUF->SBUF casts xstage -> xs8, one
    per 512-col slab, emitted two windows ahead of use.
  - scalar HWDGE ring: unused (keeps the ACT FIFO free of DMA waits).
  - x2 bias (-0.5*sum e^-logh x^2): DVE square of the fp16 slab (2x
    mode), 4 accumulating PE matmuls into a 2-bank PSUM ring, DVE row
    copy, DRAM-bounce transpose [1,512] -> [128,4].
  - ey2rep (exp(-0.5*y2) replicated to 128 partitions): one 512-wide js
    piece per itile over itiles 1-4: PE reduce -> DVE bf16 row copy ->
    ones-matmul into a main PSUM tile -> ACT exp straight into the
    fp16 [P, MJ] tile. Multiplies for itiles 0-4 are deferred until the
    last piece is emitted (program order defines dependencies).
  - Main loop per itile: 8 fp8 DoubleRow matmuls (contraction-pair
    outer) into two [P,1024] tiles of a 3-deep PSUM ring, ACT exp per
    half (bias = -0.5*x2[i], scale = -0.5), DVE fp16 multiply by
    ey2rep, store.

Host code only reshapes/transposes/shards numpy arrays, picks the fp16
staging precision, and losslessly widens the fp16 result.
"""

import json

import numpy as np

import concourse.bass as bass
import concourse.mybir as mybir
import concourse.tile as tile
from concourse.bass_utils import run_bass_kernel_spmd

N_CORES = 8
N, M, D = 8192, 8192, 512
RG, CG = 2, 4
NI = N // RG  # 4096
MJ = M // CG  # 2048
P = 128
NCHUNK = D // P  # 4
NPAIR = NCHUNK // 2  # 2
ITILES = NI // P  # 32
SLABW = 512
NSLAB = NI // SLABW  # 8
IPS = SLABW // P  # 4
HW = MJ // 2  # 1024

F32 = mybir.dt.float32
BF16 = mybir.dt.bfloat16
F16 = mybir.dt.float16
FP8 = mybir.dt.float8e4
AF = mybir.ActivationFunctionType
DR = mybir.MatmulPerfMode.DoubleRow

_WAIT_LIMIT = 1


def _split_excess_waits(bir: dict, limit: int = _WAIT_LIMIT) -> dict:
    counter = 0
    for fn in bir.get("functions", []):
        for bb in fn.get("blocks", []):
            new_insts = []
            for inst in bb.get("instructions", []):
                si = inst.get("sync_info")
                waits = si.get("on_wait") if si else None
                eng = inst.get("engine", "Unassigned")
                if waits and len(waits) > limit and eng != "Unassigned":
                    keep = len(waits) % 2
                    head = waits[: len(waits) - keep]
                    for i in range(0, len(head), 2):
                        counter += 1
                        new_insts.append(
                            {
                                "debug": inst.get("debug", 0),
                                "engine": eng,
                                "ins": [],
                                "outs": [],
                                "name": f"WS-{counter}-{inst['name']}",
                                "opcode": "EventSemaphore",
                                "sync_info": {
                                    "on_update": [],
                                    "on_wait": head[i : i + 2],
                                },
                            }
                        )
                    si["on_wait"] = waits[len(waits) - keep :]
                new_insts.append(inst)
            bb["instructions"] = new_insts
    return bir


def _patch_nc(nc):
    orig = nc.to_json_bytes

    def patched() -> bytes:
        return json.dumps(_split_excess_waits(json.loads(orig()))).encode()

    nc.to_json_bytes = patched
    return nc


def _build_nc():
    nc = bass.Bass()

    xt = nc.dram_tensor("xt", [D, NI], F16, kind="ExternalInput")
    yt = nc.dram_tensor("yt", [D, MJ], F16, kind="ExternalInput")
    lh = nc.dram_tensor("lh", [NCHUNK, P], F32, kind="ExternalInput")
    out = nc.dram_tensor("out", [NI, MJ], F16, kind="ExternalOutput")
    scratch = nc.dram_tensor("scratch", [NSLAB, SLABW], F32, kind="Internal")

    xt_r = xt.rearrange("(c d) i -> d c i", d=P)
    yt_r = yt.rearrange("(c d) j -> d c j", d=P)

    with tile.TileContext(nc) as tc:
        with (
            tc.tile_pool(name="singles", bufs=1) as singles,
            tc.tile_pool(name="sqyp", bufs=4) as sqyp,
            tc.tile_pool(name="sqxp", bufs=2) as sqxp,
            tc.tile_pool(name="sxp", bufs=2) as sxp,
            tc.tile_pool(name="yrowp", bufs=2) as yrowp,
            tc.tile_pool(name="tmpp", bufs=8) as tmpp,
            tc.tile_pool(name="outp", bufs=10) as outp,
            tc.tile_pool(name="psp", bufs=3, space="PSUM") as psp,
            tc.tile_pool(name="accp", bufs=2, space="PSUM") as accp,
        ):
            xs8 = singles.tile([P, NCHUNK, NI], FP8)
            ys8 = singles.tile([P, NCHUNK, MJ], FP8)
            mhx2 = singles.tile([P, ITILES], F32)
            ey2rep = singles.tile([P, MJ], F16)
            ones1 = singles.tile([1, P], BF16)
            lhs = singles.tile([P, NCHUNK], F32)
            emlh = singles.tile([P, NCHUNK], F32)
            ihm2 = singles.tile([P, NCHUNK], F32)
            mihsq = singles.tile([P, NCHUNK], BF16)
            ident = singles.tile([1, 1], F32)
            xstage = [
                singles.tile([P, NCHUNK * SLABW], F16, name=f"xst{s}")
                for s in range(NSLAB)
            ]
            ystage = [
                singles.tile([P, MJ], F16, name=f"yst{c}")
                for c in range(NCHUNK)
            ]

            # consts
            nc.sync.dma_start(out=lhs, in_=lh.rearrange("c d -> d c"))
            nc.scalar.activation(emlh, lhs, AF.Exp, scale=-1.0)
            nc.scalar.mul(ihm2, emlh, -2.0)
            nc.scalar.mul(mihsq, emlh, -0.5)
            nc.vector.memset(ones1, 1.0)
            nc.vector.memset(ident, 1.0)

            # head loads, shallow: y chunks then only the first 3 x slabs
            for c in range(NCHUNK):
                nc.sync.dma_start(out=ystage[c], in_=yt_r[:, c, :])
            # slab 0 goes straight to fp8 via a SWDGE DRAM cast-load on the
            # otherwise-empty gpsimd ring: ready ~10us earlier than a
            # staged load + SBUF->SBUF cast queued behind it.
            nc.gpsimd.dma_start(
                out=xs8[:, :, 0:SLABW], in_=xt_r[:, :, 0:SLABW]
            )
            for s in range(1, 3):
                nc.sync.dma_start(
                    out=xstage[s], in_=xt_r[:, :, s * SLABW : (s + 1) * SLABW]
                )

            def cast_slab(s):
                nc.gpsimd.dma_start(
                    out=xs8[:, :, s * SLABW : (s + 1) * SLABW], in_=xstage[s]
                )

            cast_slab(1)

            # x slab 0 prep: square from the fp8 values (slab 0 only)
            sqx = sqxp.tile([P, NCHUNK, SLABW], BF16, tag="sx", name="sqx0")
            nc.vector.tensor_mul(sqx, xs8[:, :, 0:SLABW], xs8[:, :, 0:SLABW])
            xacc = accp.tile([1, SLABW], F32, tag="a", name="xa0")
            for c in range(NCHUNK):
                nc.tensor.matmul(
                    xacc,
                    mihsq[:, c : c + 1],
                    sqx[:, c, :],
                    start=(c == 0),
                    stop=(c == NCHUNK - 1),
                )
            sx0 = sxp.tile([1, SLABW], F32, tag="r", name="sx0")
            nc.vector.tensor_copy(sx0, xacc)
            mht = psp.tile([P, HW], F32, tag="ps", name="mht0")
            for t in range(IPS):
                nc.tensor.transpose(
                    mht[:, t : t + 1], sx0[0:1, t * P : (t + 1) * P], ident
                )
            nc.vector.tensor_copy(mhx2[:, 0:IPS], mht[:, 0:IPS])

            # y prep: squares (DVE) + fp8 conversions (ACT)
            sqys = []
            for c in range(NCHUNK):
                sqy = sqyp.tile([P, MJ], BF16, tag="sq", name=f"sqy{c}")
                nc.vector.tensor_mul(sqy, ystage[c], ystage[c])
                sqys.append(sqy)
                nc.scalar.mul(ys8[:, c, :], ystage[c], ihm2[:, c : c + 1])

            # ---- main loop ----
            pending = []
            for it in range(ITILES):
                w = it // IPS
                k = it % IPS
                if k == 0:
                    if w + 2 < NSLAB:
                        cast_slab(w + 2)
                    if w + 3 < NSLAB:
                        nc.sync.dma_start(
                            out=xstage[w + 3],
                            in_=xt_r[:, :, (w + 3) * SLABW : (w + 4) * SLABW],
                        )
                s = w + 1
                if s < NSLAB:
                    s0 = s * SLABW
                    if k == 0:
                        sqx = sqxp.tile(
                            [P, NCHUNK * SLABW], BF16, tag="sx", name=f"sqx{s}"
                        )
                        nc.vector.tensor_mul(sqx, xstage[s], xstage[s])
                    elif k == 1:
                        xacc = accp.tile([1, SLABW], F32, tag="a", name=f"xa{s}")
                        for c in range(NCHUNK):
                            nc.tensor.matmul(
                                xacc,
                                mihsq[:, c : c + 1],
                                sqx[:, c * SLABW : (c + 1) * SLABW],
                                start=(c == 0),
                                stop=(c == NCHUNK - 1),
                            )
                        sxr = sxp.tile([1, SLABW], F32, tag="r", name=f"sx{s}")
                        nc.vector.tensor_copy(sxr, xacc)
                    elif k == 2:
                        mht = psp.tile([P, HW], F32, tag="ps", name=f"mht{s}")
                        for t in range(IPS):
                            nc.tensor.transpose(
                                mht[:, t : t + 1],
                                sxr[0:1, t * P : (t + 1) * P],
                                ident,
                            )
                        nc.vector.tensor_copy(
                            mhx2[:, s * IPS : (s + 1) * IPS], mht[:, 0:IPS]
                        )

                isl = slice(it * P, (it + 1) * P)
                tmp = tmpp.tile([P, MJ], F16, tag="tmp", name=f"t{it}")
                ps0 = psp.tile([P, HW], F32, tag="ps", name=f"ps{it}_0")
                ps1 = psp.tile([P, HW], F32, tag="ps", name=f"ps{it}_1")
                pss = (ps0, ps1)
                for t in range(NPAIR):
                    csl = slice(2 * t, 2 * t + 2)
                    for pos in range(4):
                        h, js = divmod(pos, 2)
                        nc.tensor.matmul(
                            pss[h][:, js * 512 : (js + 1) * 512],
                            xs8[:, csl, isl],
                            ys8[:, csl, pos * 512 : (pos + 1) * 512],
                            start=(t == 0),
                            stop=(t == NPAIR - 1),
                            perf_mode=DR,
                        )
                for h in range(2):
                    nc.scalar.activation(
                        tmp[:, h * HW : (h + 1) * HW],
                        pss[h],
                        AF.Exp,
                        bias=mhx2[:, it : it + 1],
                        scale=-0.5,
                    )

                # ey2rep piece js = it-1 over itiles 1..4 (PE/DVE/ACT filler)
                if 1 <= it <= MJ // 512:
                    js = it - 1
                    jsl = slice(js * 512, (js + 1) * 512)
                    yacc = accp.tile([1, 512], F32, tag="a", name=f"ya{js}")
                    for c in range(NCHUNK):
                        nc.tensor.matmul(
                            yacc,
                            mihsq[:, c : c + 1],
                            sqys[c][:, jsl],
                            start=(c == 0),
                            stop=(c == NCHUNK - 1),
                        )
                    yrow = yrowp.tile([1, 512], BF16, tag="yr", name=f"yr{js}")
                    nc.vector.tensor_copy(yrow, yacc)
                    rep = psp.tile([P, HW], F32, tag="ps", name=f"rep{js}")
                    nc.tensor.matmul(
                        rep[:, 0:512], ones1, yrow, start=True, stop=True
                    )
                    nc.scalar.activation(ey2rep[:, jsl], rep[:, 0:512], AF.Exp)

                # multiplies/stores; itiles 0-4 deferred until the last
                # ey2rep piece's writers exist in program order
                pending.append((tmp, isl, it))
                if it >= MJ // 512:
                    for ptmp, pisl, pit in pending:
                        ot = outp.tile([P, MJ], F16, tag="ot", name=f"ot{pit}")
                        nc.vector.tensor_mul(ot, ptmp, ey2rep)
                        nc.sync.dma_start(out=out[pisl, :], in_=ot)
                    pending.clear()

    return _patch_nc(nc)


_NC_CACHE = None
_TRACE = False
_TRACE_KWARGS = {}
LAST_RESULTS = None


def kernel(x, y, logh):
    global _NC_CACHE, LAST_RESULTS
    x = np.ascontiguousarray(np.asarray(x, dtype=np.float32))
    y = np.ascontiguousarray(np.asarray(y, dtype=np.float32))
    logh = np.ascontiguousarray(np.asarray(logh, dtype=np.float32))
    assert x.shape == (N, D) and y.shape == (M, D) and logh.shape == (D,)

    if _NC_CACHE is None:
        _NC_CACHE = _build_nc()
    nc = _NC_CACHE

    lhm = np.ascontiguousarray(logh.reshape(NCHUNK, P))
    x16 = x.astype(np.float16)
    y16 = y.astype(np.float16)
    xts = [
        np.ascontiguousarray(x16[r * NI : (r + 1) * NI, :].T)
        for r in range(RG)
    ]
    yts = [
        np.ascontiguousarray(y16[q * MJ : (q + 1) * MJ, :].T)
        for q in range(CG)
    ]
    in_maps = []
    for c in range(N_CORES):
        r, q = divmod(c, CG)
        in_maps.append({"xt": xts[r], "yt": yts[q], "lh": lhm})

    res = run_bass_kernel_spmd(
        nc,
        in_maps,
        core_ids=list(range(N_CORES)),
        trace=_TRACE,
        **_TRACE_KWARGS,
    )
    LAST_RESULTS = res
    full = np.empty((N, M), dtype=np.float32)
    for c in range(N_CORES):
        r, q = divmod(c, CG)
        full[r * NI : (r + 1) * NI, q * MJ : (q + 1) * MJ] = res.results[c][
            "out"
        ].astype(np.float32)
    return full
